# revision 2
# baseline (speedup 1.0000x reference)
"""Trainium2 Bass kernel for the torchhd-style MNIST HDC encoder model.

Computation (see reference):
    idx   = clip(round(x.reshape(B, P) * 255), 0, 255)            # [B, P] ints
    bund  = sum_p position[p, :] * level_weight[idx[b, p], :]     # [B, D]
    enc   = where(bund > 0, 1, -1)                                # [B, D]
    logit = enc @ classify_weight.T                               # [B, C]

Strategy: shard the hypervector dimension D=10000 across 8 cores (1250 cols
each, zero-padded to 1280).  Per core and per batch image:
  - dma_gather pulls the 784 indexed level rows (bf16, exact for +-1) from
    HBM into SBUF, laid out [128 part, 7 blk, 1280].
  - DVE multiplies elementwise with the identically-laid-out position rows.
  - TensorE contracts the 784 pixel rows with a one-hot batch-selector
    lhsT so each image's bundled row accumulates into its own PSUM
    partition (fp32, exact integer sums).
  - ACT applies sign(x - 0.5), TensorE transposes and applies the classify
    matmul in fp32; each core emits partial logits [10, 64] over its D
    chunk, summed on the host.
"""

import os
import sys

for _p in ("/opt/trn_rl_repo", "/root/.axon_site/_ro/trn_rl_repo"):
    if os.path.isdir(_p) and _p not in sys.path:
        sys.path.insert(0, _p)

import ml_dtypes
import numpy as np

BATCH = 64
P = 784            # 28*28 pixels
D = 10000          # hypervector dim
L = 256            # quantization levels
C = 10             # classes
NCORES = 8
DC = D // NCORES   # 1250 real cols per core
DP = 1280          # padded cols (2560B bf16 rows: dma_gather needs %256B)
PBLK = 7           # ceil(784/128) partition blocks of gathered rows
PTAIL = P - 6 * 128  # 16 rows in the last block

_compiled = None   # (nc, run_kwargs) cache


def _build_bass():
    import concourse.bacc as bacc
    import concourse.tile as tile
    from concourse import mybir

    fp32 = mybir.dt.float32
    bf16 = mybir.dt.bfloat16
    i16 = mybir.dt.int16

    nc = bacc.Bacc("TRN2", target_bir_lowering=False, debug=False,
                   enable_asserts=False)

    # DRAM I/O (per-core arrays supplied at run time)
    lvl = nc.dram_tensor("lvl", [L, DP], bf16, kind="ExternalInput")
    posw = nc.dram_tensor("posw", [128, PBLK * DP], bf16, kind="ExternalInput")
    selw = nc.dram_tensor("selw", [128, BATCH * BATCH], bf16, kind="ExternalInput")
    clsw = nc.dram_tensor("clsw", [128, (DP // 128) * C], fp32, kind="ExternalInput")
    idxw = nc.dram_tensor("idxw", [128, BATCH * (P // 16)], i16, kind="ExternalInput")
    identw = nc.dram_tensor("identw", [BATCH, BATCH], bf16, kind="ExternalInput")
    out = nc.dram_tensor("logitT", [C, BATCH], fp32, kind="ExternalOutput")

    NIDX = P // 16        # 49 idx columns per image
    KT = DP // 128        # 10 classify contraction tiles
    CHUNKS = [(0, 512), (512, 512), (1024, DP - 1024)]  # psum-bank chunks

    with tile.TileContext(nc) as tc:
        with (
            tc.tile_pool(name="const", bufs=1) as cpool,
            tc.tile_pool(name="gath", bufs=1) as gpool,
            tc.tile_pool(name="prod", bufs=1) as ppool,
            tc.tile_pool(name="misc", bufs=1) as mpool,
            tc.tile_pool(name="psum", bufs=1, space="PSUM") as psum,
            tc.tile_pool(name="psumt", bufs=2, space="PSUM") as psumt,
        ):
            pos_sb = cpool.tile([128, PBLK * DP], bf16)
            nc.sync.dma_start(pos_sb[:], posw.ap())
            sel_sb = cpool.tile([128, BATCH * BATCH], bf16)
            nc.sync.dma_start(sel_sb[:], selw.ap())
            cls_sb = cpool.tile([128, KT * C], fp32)
            nc.sync.dma_start(cls_sb[:], clsw.ap())
            idx_sb = cpool.tile([128, BATCH * NIDX], i16)
            nc.sync.dma_start(idx_sb[:], idxw.ap())
            id_sb = cpool.tile([BATCH, BATCH], bf16)
            nc.sync.dma_start(id_sb[:], identw.ap())

            bund = psum.tile([BATCH, DP], fp32)

            # double-buffered gather/product tiles, rotated manually
            g_tiles = [gpool.tile([128, PBLK * DP], bf16, name=f"g{i}")
                       for i in range(2)]
            pr_tiles = [ppool.tile([128, PBLK * DP], bf16, name=f"pr{i}")
                        for i in range(2)]

            for b in range(BATCH):
                g = g_tiles[b % 2]
                pr = pr_tiles[b % 2]
                g3 = g[:].rearrange("p (n m) -> p n m", m=DP)
                pr3 = pr[:].rearrange("p (n m) -> p n m", m=DP)

                nc.gpsimd.dma_gather(
                    g3, lvl.ap(), idx_sb[:, b * NIDX:(b + 1) * NIDX],
                    num_idxs=P, num_idxs_reg=P, elem_size=DP,
                )
                # bind: full 6 blocks on all 128 partitions, last block only
                # has PTAIL valid rows
                nc.vector.tensor_mul(pr[:, :6 * DP], g[:, :6 * DP],
                                     pos_sb[:, :6 * DP])
                nc.vector.tensor_mul(pr[:PTAIL, 6 * DP:], g[:PTAIL, 6 * DP:],
                                     pos_sb[:PTAIL, 6 * DP:])

                sel_b = sel_sb[:, b * BATCH:(b + 1) * BATCH]
                for pt in range(PBLK):
                    kp = 128 if pt < 6 else PTAIL
                    for (c0, cn) in CHUNKS:
                        nc.tensor.matmul(
                            bund[:, c0:c0 + cn],
                            sel_b[:kp, :],
                            pr3[:kp, pt, c0:c0 + cn],
                            start=(b == 0 and pt == 0),
                            stop=(b == BATCH - 1 and pt == PBLK - 1),
                        )

            # sign (integer sums; -0.5 bias makes where(x>0,1,-1) exact)
            bias_t = mpool.tile([BATCH, 1], fp32)
            nc.gpsimd.memset(bias_t[:], -0.5)
            enc = mpool.tile([BATCH, DP], bf16)
            nc.scalar.activation(enc[:], bund[:],
                                 mybir.ActivationFunctionType.Sign,
                                 bias=bias_t[:])

            # classify: transpose 128-col chunks of enc, then fp32 matmul
            logit_ps = psum.tile([C, BATCH], fp32)
            for kt in range(KT):
                tp = psumt.tile([128, BATCH], bf16, name="tp")
                nc.tensor.transpose(tp[:], enc[:, kt * 128:(kt + 1) * 128],
                                    id_sb[:])
                etc = mpool.tile([128, BATCH], fp32, name="etc", bufs=2)
                nc.scalar.copy(etc[:], tp[:])
                nc.tensor.matmul(
                    logit_ps[:], cls_sb[:, kt * C:(kt + 1) * C], etc[:],
                    start=(kt == 0), stop=(kt == KT - 1),
                )

            logit_sb = mpool.tile([C, BATCH], fp32)
            nc.scalar.copy(logit_sb[:], logit_ps[:])
            nc.sync.dma_start(out.ap(), logit_sb[:])

    nc.compile()
    return nc


def _prep_inputs(x, position, level_weight, classify_weight):
    """Host-side shard prep: returns in_maps for the 8 cores."""
    xf = x.reshape(BATCH, P).astype(np.float32)
    idx = np.clip(np.round(xf * np.float32(L - 1)), 0, L - 1).astype(np.int16)
    # dma_gather wraps indices as [16, n/16]: index j at [j%16, j//16],
    # replicated across all 128 partitions
    idxw = np.ascontiguousarray(
        idx.reshape(BATCH, P // 16, 16).transpose(2, 0, 1)
    ).reshape(16, BATCH * (P // 16))
    idxw = np.tile(idxw, (8, 1))  # [128, ...]

    sel = np.zeros((128, BATCH * BATCH), np.float32)
    for b in range(BATCH):
        sel[:, b * BATCH + b] = 1.0
    selw = sel.astype(ml_dtypes.bfloat16)

    identw = np.eye(BATCH, dtype=np.float32).astype(ml_dtypes.bfloat16)

    KT = DP // 128
    in_maps = []
    for core in range(NCORES):
        cols = slice(core * DC, (core + 1) * DC)

        lvl = np.zeros((L, DP), ml_dtypes.bfloat16)
        lvl[:, :DC] = level_weight[:, cols].astype(ml_dtypes.bfloat16)

        pos = np.zeros((6 * 128 + 128, DP), np.float32)
        pos[:P, :DC] = position[:, cols]
        posw = np.ascontiguousarray(
            pos.reshape(PBLK, 128, DP).transpose(1, 0, 2)
        ).reshape(128, PBLK * DP).astype(ml_dtypes.bfloat16)

        cls = np.zeros((C, DP), np.float32)
        cls[:, :DC] = classify_weight[:, cols]
        # lhsT chunks: clsw[p, kt*C + m] = cls[m, kt*128 + p]
        clsw = np.ascontiguousarray(
            cls.reshape(C, KT, 128).transpose(2, 1, 0)
        ).reshape(128, KT * C)

        in_maps.append({
            "lvl": lvl,
            "posw": posw,
            "selw": selw,
            "clsw": clsw,
            "idxw": idxw,
            "identw": identw,
        })
    return in_maps


def kernel(x, position, level_weight, classify_weight, _run_kwargs=None):
    global _compiled
    if _compiled is None:
        _compiled = _build_bass()
    nc = _compiled

    import concourse.bass_utils as bass_utils

    in_maps = _prep_inputs(x, position, level_weight, classify_weight)
    res = bass_utils.run_bass_kernel_spmd(
        nc, in_maps, core_ids=list(range(NCORES)), **(_run_kwargs or {})
    )
    logit = np.zeros((BATCH, C), np.float32)
    for core in range(NCORES):
        logit += res.results[core]["logitT"].T.astype(np.float32)
    kernel.last_result = res
    return logit


# revision 5
# speedup vs baseline: 1.4327x; 1.4327x over previous
"""Trainium2 Bass kernel for the torchhd-style MNIST HDC encoder model.

Computation (see reference):
    idx   = clip(round(x.reshape(B, P) * 255), 0, 255)            # [B, P] ints
    bund  = sum_p position[p, :] * level_weight[idx[b, p], :]     # [B, D]
    enc   = where(bund > 0, 1, -1)                                # [B, D]
    logit = enc @ classify_weight.T                               # [B, C]

Strategy: shard the hypervector dimension D=10000 across 8 cores (1250 cols
each, zero-padded to 1280).  Per core and per batch image:
  - dma_gather pulls the 784 indexed level rows (bf16, exact for +-1) from
    HBM into SBUF, laid out [128 part, 7 blk, 1280].
  - DVE multiplies elementwise with the identically-laid-out position rows.
  - TensorE contracts the 784 pixel rows with a one-hot batch-selector
    lhsT so each image's bundled row accumulates into its own PSUM
    partition (fp32, exact integer sums).
  - ACT applies sign(x - 0.5), TensorE transposes and applies the classify
    matmul in fp32; each core emits partial logits [10, 64] over its D
    chunk, summed on the host.
"""

import os
import sys

for _p in ("/opt/trn_rl_repo", "/root/.axon_site/_ro/trn_rl_repo"):
    if os.path.isdir(_p) and _p not in sys.path:
        sys.path.insert(0, _p)

import ml_dtypes
import numpy as np

BATCH = 64
P = 784            # 28*28 pixels
D = 10000          # hypervector dim
L = 256            # quantization levels
C = 10             # classes
NCORES = 8
DC = D // NCORES   # 1250 real cols per core
DP = 1280          # padded cols (2560B bf16 rows: dma_gather needs %256B)
PBLK = 7           # ceil(784/128) partition blocks of gathered rows
PTAIL = P - 6 * 128  # 16 rows in the last block

_compiled = None   # (nc, run_kwargs) cache


def _build_bass():
    import concourse.bacc as bacc
    import concourse.tile as tile
    from concourse import mybir

    fp32 = mybir.dt.float32
    bf16 = mybir.dt.bfloat16
    i16 = mybir.dt.int16

    nc = bacc.Bacc("TRN2", target_bir_lowering=False, debug=False,
                   enable_asserts=False, num_swdge_queues=4)

    # DRAM I/O (per-core arrays supplied at run time)
    lvl = nc.dram_tensor("lvl", [L, DP], bf16, kind="ExternalInput")
    posw = nc.dram_tensor("posw", [128, PBLK * DP], bf16, kind="ExternalInput")
    selw = nc.dram_tensor("selw", [128, BATCH * BATCH], bf16, kind="ExternalInput")
    clsw = nc.dram_tensor("clsw", [128, (DP // 128) * C], fp32, kind="ExternalInput")
    idxw = nc.dram_tensor("idxw", [128, BATCH * (P // 16)], i16, kind="ExternalInput")
    identw = nc.dram_tensor("identw", [BATCH, BATCH], bf16, kind="ExternalInput")
    out = nc.dram_tensor("logitT", [C, BATCH], fp32, kind="ExternalOutput")

    NIDX = P // 16        # 49 idx columns per image
    KT = DP // 128        # 10 classify contraction tiles
    CHUNKS = [(0, 512), (512, 512), (1024, DP - 1024)]  # psum-bank chunks

    with tile.TileContext(nc) as tc:
        with (
            tc.tile_pool(name="const", bufs=1) as cpool,
            tc.tile_pool(name="gath", bufs=1) as gpool,
            tc.tile_pool(name="prod", bufs=1) as ppool,
            tc.tile_pool(name="misc", bufs=1) as mpool,
            tc.tile_pool(name="psum", bufs=1, space="PSUM") as psum,
            tc.tile_pool(name="psumt", bufs=2, space="PSUM") as psumt,
        ):
            pos_sb = cpool.tile([128, PBLK * DP], bf16)
            nc.sync.dma_start(pos_sb[:], posw.ap())
            sel_sb = cpool.tile([128, BATCH * BATCH], bf16)
            nc.sync.dma_start(sel_sb[:], selw.ap())
            cls_sb = cpool.tile([128, KT * C], fp32)
            nc.sync.dma_start(cls_sb[:], clsw.ap())
            idx_sb = cpool.tile([128, BATCH * NIDX], i16)
            nc.sync.dma_start(idx_sb[:], idxw.ap())
            id_sb = cpool.tile([BATCH, BATCH], bf16)
            nc.sync.dma_start(id_sb[:], identw.ap())

            bund = psum.tile([BATCH, DP], fp32)

            # multi-buffered gather/product tiles, rotated manually
            NBUF = 3
            g_tiles = [gpool.tile([128, PBLK * DP], bf16, name=f"g{i}")
                       for i in range(NBUF)]
            pr_tiles = [ppool.tile([128, PBLK * DP], bf16, name=f"pr{i}")
                        for i in range(NBUF)]

            for b in range(BATCH):
                g = g_tiles[b % NBUF]
                pr = pr_tiles[b % NBUF]
                g3 = g[:].rearrange("p (n m) -> p n m", m=DP)
                pr3 = pr[:].rearrange("p (n m) -> p n m", m=DP)

                nc.gpsimd.dma_gather(
                    g3, lvl.ap(), idx_sb[:, b * NIDX:(b + 1) * NIDX],
                    num_idxs=P, num_idxs_reg=P, elem_size=DP,
                    queue_num=b % 4,
                )
                # bind: full 6 blocks on all 128 partitions, last block only
                # has PTAIL valid rows
                nc.vector.tensor_mul(pr[:, :6 * DP], g[:, :6 * DP],
                                     pos_sb[:, :6 * DP])
                nc.vector.tensor_mul(pr[:PTAIL, 6 * DP:], g[:PTAIL, 6 * DP:],
                                     pos_sb[:PTAIL, 6 * DP:])

                sel_b = sel_sb[:, b * BATCH:(b + 1) * BATCH]
                for pt in range(PBLK):
                    kp = 128 if pt < 6 else PTAIL
                    for (c0, cn) in CHUNKS:
                        nc.tensor.matmul(
                            bund[:, c0:c0 + cn],
                            sel_b[:kp, :],
                            pr3[:kp, pt, c0:c0 + cn],
                            start=(b == 0 and pt == 0),
                            stop=(b == BATCH - 1 and pt == PBLK - 1),
                        )

            # sign (integer sums; -0.5 bias makes where(x>0,1,-1) exact)
            bias_t = mpool.tile([BATCH, 1], fp32)
            nc.gpsimd.memset(bias_t[:], -0.5)
            enc = mpool.tile([BATCH, DP], bf16)
            nc.scalar.activation(enc[:], bund[:],
                                 mybir.ActivationFunctionType.Sign,
                                 bias=bias_t[:])

            # classify: transpose 128-col chunks of enc, then fp32 matmul
            logit_ps = psum.tile([C, BATCH], fp32)
            for kt in range(KT):
                tp = psumt.tile([128, BATCH], bf16, name="tp")
                nc.tensor.transpose(tp[:], enc[:, kt * 128:(kt + 1) * 128],
                                    id_sb[:])
                etc = mpool.tile([128, BATCH], fp32, name="etc", bufs=2)
                nc.scalar.copy(etc[:], tp[:])
                nc.tensor.matmul(
                    logit_ps[:], cls_sb[:, kt * C:(kt + 1) * C], etc[:],
                    start=(kt == 0), stop=(kt == KT - 1),
                )

            logit_sb = mpool.tile([C, BATCH], fp32)
            nc.scalar.copy(logit_sb[:], logit_ps[:])
            nc.sync.dma_start(out.ap(), logit_sb[:])

    nc.compile()
    return nc


def _prep_inputs(x, position, level_weight, classify_weight):
    """Host-side shard prep: returns in_maps for the 8 cores."""
    xf = x.reshape(BATCH, P).astype(np.float32)
    idx = np.clip(np.round(xf * np.float32(L - 1)), 0, L - 1).astype(np.int16)
    # dma_gather wraps indices as [16, n/16]: index j at [j%16, j//16],
    # replicated across all 128 partitions
    idxw = np.ascontiguousarray(
        idx.reshape(BATCH, P // 16, 16).transpose(2, 0, 1)
    ).reshape(16, BATCH * (P // 16))
    idxw = np.tile(idxw, (8, 1))  # [128, ...]

    sel = np.zeros((128, BATCH * BATCH), np.float32)
    for b in range(BATCH):
        sel[:, b * BATCH + b] = 1.0
    selw = sel.astype(ml_dtypes.bfloat16)

    identw = np.eye(BATCH, dtype=np.float32).astype(ml_dtypes.bfloat16)

    KT = DP // 128
    in_maps = []
    for core in range(NCORES):
        cols = slice(core * DC, (core + 1) * DC)

        lvl = np.zeros((L, DP), ml_dtypes.bfloat16)
        lvl[:, :DC] = level_weight[:, cols].astype(ml_dtypes.bfloat16)

        pos = np.zeros((6 * 128 + 128, DP), np.float32)
        pos[:P, :DC] = position[:, cols]
        posw = np.ascontiguousarray(
            pos.reshape(PBLK, 128, DP).transpose(1, 0, 2)
        ).reshape(128, PBLK * DP).astype(ml_dtypes.bfloat16)

        cls = np.zeros((C, DP), np.float32)
        cls[:, :DC] = classify_weight[:, cols]
        # lhsT chunks: clsw[p, kt*C + m] = cls[m, kt*128 + p]
        clsw = np.ascontiguousarray(
            cls.reshape(C, KT, 128).transpose(2, 1, 0)
        ).reshape(128, KT * C)

        in_maps.append({
            "lvl": lvl,
            "posw": posw,
            "selw": selw,
            "clsw": clsw,
            "idxw": idxw,
            "identw": identw,
        })
    return in_maps


def kernel(x, position, level_weight, classify_weight, _run_kwargs=None):
    global _compiled
    if _compiled is None:
        _compiled = _build_bass()
    nc = _compiled

    import concourse.bass_utils as bass_utils

    in_maps = _prep_inputs(x, position, level_weight, classify_weight)
    res = bass_utils.run_bass_kernel_spmd(
        nc, in_maps, core_ids=list(range(NCORES)), **(_run_kwargs or {})
    )
    logit = np.zeros((BATCH, C), np.float32)
    for core in range(NCORES):
        logit += res.results[core]["logitT"].T.astype(np.float32)
    kernel.last_result = res
    return logit


# revision 6
# speedup vs baseline: 1.5385x; 1.0738x over previous
"""Trainium2 Bass kernel for the torchhd-style MNIST HDC encoder model.

Computation (see reference):
    idx   = clip(round(x.reshape(B, P) * 255), 0, 255)            # [B, P] ints
    bund  = sum_p position[p, :] * level_weight[idx[b, p], :]     # [B, D]
    enc   = where(bund > 0, 1, -1)                                # [B, D]
    logit = enc @ classify_weight.T                               # [B, C]

Strategy: shard the hypervector dimension D=10000 across 8 cores (1250 cols
each, zero-padded to 1280).  Per core and per batch image:
  - dma_gather pulls the 784 indexed level rows (bf16, exact for +-1) from
    HBM into SBUF, laid out [128 part, 7 blk, 1280].
  - DVE multiplies elementwise with the identically-laid-out position rows.
  - TensorE contracts the 784 pixel rows with a one-hot batch-selector
    lhsT so each image's bundled row accumulates into its own PSUM
    partition (fp32, exact integer sums).
  - ACT applies sign(x - 0.5), TensorE transposes and applies the classify
    matmul in fp32; each core emits partial logits [10, 64] over its D
    chunk, summed on the host.
"""

import os
import sys

for _p in ("/opt/trn_rl_repo", "/root/.axon_site/_ro/trn_rl_repo"):
    if os.path.isdir(_p) and _p not in sys.path:
        sys.path.insert(0, _p)

import ml_dtypes
import numpy as np

BATCH = 64
P = 784            # 28*28 pixels
D = 10000          # hypervector dim
L = 256            # quantization levels
C = 10             # classes
NCORES = 8
DC = D // NCORES   # 1250 real cols per core
DP = 1280          # padded cols (2560B bf16 rows: dma_gather needs %256B)
PBLK = 7           # ceil(784/128) partition blocks of gathered rows
PTAIL = P - 6 * 128  # 16 rows in the last block

_compiled = None   # (nc, run_kwargs) cache


def _build_bass():
    import concourse.bacc as bacc
    import concourse.tile as tile
    from concourse import mybir

    fp32 = mybir.dt.float32
    bf16 = mybir.dt.bfloat16
    i16 = mybir.dt.int16

    nc = bacc.Bacc("TRN2", target_bir_lowering=False, debug=False,
                   enable_asserts=False, num_swdge_queues=4)

    # DRAM I/O (per-core arrays supplied at run time)
    lvl = nc.dram_tensor("lvl", [L, DP], bf16, kind="ExternalInput")
    posw = nc.dram_tensor("posw", [128, PBLK * DP], bf16, kind="ExternalInput")
    selw = nc.dram_tensor("selw", [128, BATCH * BATCH], bf16, kind="ExternalInput")
    clsw = nc.dram_tensor("clsw", [128, (DP // 128) * C], fp32, kind="ExternalInput")
    idxw = nc.dram_tensor("idxw", [128, BATCH * (P // 16)], i16, kind="ExternalInput")
    identw = nc.dram_tensor("identw", [BATCH, BATCH], bf16, kind="ExternalInput")
    out = nc.dram_tensor("logitT", [C, BATCH], fp32, kind="ExternalOutput")

    NIDX = P // 16        # 49 idx columns per image
    KT = DP // 128        # 10 classify contraction tiles
    CHUNKS = [(0, 512), (512, 512), (1024, DP - 1024)]  # psum-bank chunks

    with tile.TileContext(nc) as tc:
        with (
            tc.tile_pool(name="const", bufs=1) as cpool,
            tc.tile_pool(name="gath", bufs=1) as gpool,
            tc.tile_pool(name="prod", bufs=1) as ppool,
            tc.tile_pool(name="misc", bufs=1) as mpool,
            tc.tile_pool(name="psum", bufs=1, space="PSUM") as psum,
            tc.tile_pool(name="psumt", bufs=2, space="PSUM") as psumt,
        ):
            pos_sb = cpool.tile([128, PBLK * DP], bf16)
            nc.sync.dma_start(pos_sb[:], posw.ap())
            sel_sb = cpool.tile([128, BATCH * BATCH], bf16)
            nc.sync.dma_start(sel_sb[:], selw.ap())
            cls_sb = cpool.tile([128, KT * C], fp32)
            nc.sync.dma_start(cls_sb[:], clsw.ap())
            idx_sb = cpool.tile([128, BATCH * NIDX], i16)
            nc.sync.dma_start(idx_sb[:], idxw.ap())
            id_sb = cpool.tile([BATCH, BATCH], bf16)
            nc.sync.dma_start(id_sb[:], identw.ap())

            bund = psum.tile([BATCH, DP], fp32)

            # multi-buffered gather/product tiles, rotated manually
            NBUF = 4
            g_tiles = [gpool.tile([128, PBLK * DP], bf16, name=f"g{i}")
                       for i in range(NBUF)]
            pr_tiles = [ppool.tile([128, PBLK * DP], bf16, name=f"pr{i}")
                        for i in range(NBUF)]

            for b in range(BATCH):
                g = g_tiles[b % NBUF]
                pr = pr_tiles[b % NBUF]
                g3 = g[:].rearrange("p (n m) -> p n m", m=DP)
                pr3 = pr[:].rearrange("p (n m) -> p n m", m=DP)

                nc.gpsimd.dma_gather(
                    g3, lvl.ap(), idx_sb[:, b * NIDX:(b + 1) * NIDX],
                    num_idxs=P, num_idxs_reg=P, elem_size=DP,
                    queue_num=b % 4,
                )
                # bind: full 6 blocks on all 128 partitions, last block only
                # has PTAIL valid rows
                nc.vector.tensor_mul(pr[:, :6 * DP], g[:, :6 * DP],
                                     pos_sb[:, :6 * DP])
                nc.vector.tensor_mul(pr[:PTAIL, 6 * DP:], g[:PTAIL, 6 * DP:],
                                     pos_sb[:PTAIL, 6 * DP:])

                sel_b = sel_sb[:, b * BATCH:(b + 1) * BATCH]
                for pt in range(PBLK):
                    kp = 128 if pt < 6 else PTAIL
                    for (c0, cn) in CHUNKS:
                        nc.tensor.matmul(
                            bund[:, c0:c0 + cn],
                            sel_b[:kp, :],
                            pr3[:kp, pt, c0:c0 + cn],
                            start=(b == 0 and pt == 0),
                            stop=(b == BATCH - 1 and pt == PBLK - 1),
                        )

            # sign (integer sums; -0.5 bias makes where(x>0,1,-1) exact)
            bias_t = mpool.tile([BATCH, 1], fp32)
            nc.gpsimd.memset(bias_t[:], -0.5)
            enc = mpool.tile([BATCH, DP], bf16)
            nc.scalar.activation(enc[:], bund[:],
                                 mybir.ActivationFunctionType.Sign,
                                 bias=bias_t[:])

            # classify: transpose 128-col chunks of enc, then fp32 matmul
            logit_ps = psum.tile([C, BATCH], fp32)
            for kt in range(KT):
                tp = psumt.tile([128, BATCH], bf16, name="tp")
                nc.tensor.transpose(tp[:], enc[:, kt * 128:(kt + 1) * 128],
                                    id_sb[:])
                etc = mpool.tile([128, BATCH], fp32, name="etc", bufs=2)
                nc.scalar.copy(etc[:], tp[:])
                nc.tensor.matmul(
                    logit_ps[:], cls_sb[:, kt * C:(kt + 1) * C], etc[:],
                    start=(kt == 0), stop=(kt == KT - 1),
                )

            logit_sb = mpool.tile([C, BATCH], fp32)
            nc.scalar.copy(logit_sb[:], logit_ps[:])
            nc.sync.dma_start(out.ap(), logit_sb[:])

    nc.compile()
    return nc


def _prep_inputs(x, position, level_weight, classify_weight):
    """Host-side shard prep: returns in_maps for the 8 cores."""
    xf = x.reshape(BATCH, P).astype(np.float32)
    idx = np.clip(np.round(xf * np.float32(L - 1)), 0, L - 1).astype(np.int16)
    # dma_gather wraps indices as [16, n/16]: index j at [j%16, j//16],
    # replicated across all 128 partitions
    idxw = np.ascontiguousarray(
        idx.reshape(BATCH, P // 16, 16).transpose(2, 0, 1)
    ).reshape(16, BATCH * (P // 16))
    idxw = np.tile(idxw, (8, 1))  # [128, ...]

    sel = np.zeros((128, BATCH * BATCH), np.float32)
    for b in range(BATCH):
        sel[:, b * BATCH + b] = 1.0
    selw = sel.astype(ml_dtypes.bfloat16)

    identw = np.eye(BATCH, dtype=np.float32).astype(ml_dtypes.bfloat16)

    KT = DP // 128
    in_maps = []
    for core in range(NCORES):
        cols = slice(core * DC, (core + 1) * DC)

        lvl = np.zeros((L, DP), ml_dtypes.bfloat16)
        lvl[:, :DC] = level_weight[:, cols].astype(ml_dtypes.bfloat16)

        pos = np.zeros((6 * 128 + 128, DP), np.float32)
        pos[:P, :DC] = position[:, cols]
        posw = np.ascontiguousarray(
            pos.reshape(PBLK, 128, DP).transpose(1, 0, 2)
        ).reshape(128, PBLK * DP).astype(ml_dtypes.bfloat16)

        cls = np.zeros((C, DP), np.float32)
        cls[:, :DC] = classify_weight[:, cols]
        # lhsT chunks: clsw[p, kt*C + m] = cls[m, kt*128 + p]
        clsw = np.ascontiguousarray(
            cls.reshape(C, KT, 128).transpose(2, 1, 0)
        ).reshape(128, KT * C)

        in_maps.append({
            "lvl": lvl,
            "posw": posw,
            "selw": selw,
            "clsw": clsw,
            "idxw": idxw,
            "identw": identw,
        })
    return in_maps


def kernel(x, position, level_weight, classify_weight, _run_kwargs=None):
    global _compiled
    if _compiled is None:
        _compiled = _build_bass()
    nc = _compiled

    import concourse.bass_utils as bass_utils

    in_maps = _prep_inputs(x, position, level_weight, classify_weight)
    res = bass_utils.run_bass_kernel_spmd(
        nc, in_maps, core_ids=list(range(NCORES)), **(_run_kwargs or {})
    )
    logit = np.zeros((BATCH, C), np.float32)
    for core in range(NCORES):
        logit += res.results[core]["logitT"].T.astype(np.float32)
    kernel.last_result = res
    return logit


# revision 10
# speedup vs baseline: 2.6784x; 1.7409x over previous
"""Trainium2 Bass kernel for the torchhd-style MNIST HDC encoder model.

Computation (see reference):
    idx   = clip(round(x.reshape(B, P) * 255), 0, 255)            # [B, P] ints
    bund  = sum_p position[p, :] * level_weight[idx[b, p], :]     # [B, D]
    enc   = where(bund > 0, 1, -1)                                # [B, D]
    logit = enc @ classify_weight.T                               # [B, C]

Strategy: shard the hypervector dimension D=10000 across 8 cores (1250 cols
each, zero-padded to 1280).  Everything on the bind/bundle path is +-1, so
it is carried in fp8 (exact) and the bind multiply degenerates to a sign
flip.  Per core, per batch image:
  - dma_gather pulls the 784 indexed fp8 level rows from HBM into SBUF,
    row r landing on partition r%128, block r//128 (8 blocks, zero-padded).
  - The bind pos*lvl is a bitwise XOR of the position SIGN bits into the
    gathered fp8 bytes, done on DVE over uint16-viewed data (2x mode).
  - TensorE contracts pixel rows with a one-hot batch-selector lhsT in fp8
    DoubleRow mode (256 contraction rows per matmul), accumulating each
    image's bundled row into its own PSUM partition (fp32, exact ints).
  - ACT applies sign(x - 0.5); TensorE transposes and runs the classify
    matmul in fp32; each core emits partial logits [10, 64] over its D
    chunk, summed on the host.
"""

import os
import sys

for _p in ("/opt/trn_rl_repo", "/root/.axon_site/_ro/trn_rl_repo"):
    if os.path.isdir(_p) and _p not in sys.path:
        sys.path.insert(0, _p)

import ml_dtypes
import numpy as np

BATCH = 64
P = 784            # 28*28 pixels
D = 10000          # hypervector dim
L = 256            # quantization levels
C = 10             # classes
NCORES = 8
DC = D // NCORES   # 1250 real cols per core
DP = 1280          # padded cols (1280B fp8 rows: dma_gather needs %256B)
PBLK = 8           # padded partition blocks of gathered rows (1024 slots)
GBLK = 7           # blocks actually touched by the gather (ceil(784/128))
PTAIL = P - 6 * 128  # 16 valid rows in block 6

_compiled = None


def _build_bass():
    import concourse.bacc as bacc
    import concourse.tile as tile
    from concourse import mybir

    fp32 = mybir.dt.float32
    bf16 = mybir.dt.bfloat16
    fp8 = mybir.dt.float8e4
    u16 = mybir.dt.uint16
    i16 = mybir.dt.int16

    nc = bacc.Bacc("TRN2", target_bir_lowering=False, debug=False,
                   enable_asserts=False, num_swdge_queues=4)

    # DRAM I/O (per-core arrays supplied at run time)
    lvl = nc.dram_tensor("lvl", [L, DP], fp8, kind="ExternalInput")
    posx = nc.dram_tensor("posx", [128, PBLK * DP // 2], u16,
                          kind="ExternalInput")
    selw = nc.dram_tensor("selw", [128, BATCH * 2 * BATCH], fp8,
                          kind="ExternalInput")
    clsw = nc.dram_tensor("clsw", [128, (DP // 128) * C], fp32,
                          kind="ExternalInput")
    idxw = nc.dram_tensor("idxw", [128, BATCH * (P // 16)], i16,
                          kind="ExternalInput")
    identw = nc.dram_tensor("identw", [BATCH, BATCH], bf16,
                            kind="ExternalInput")
    out = nc.dram_tensor("logitT", [C, BATCH], fp32, kind="ExternalOutput")

    NIDX = P // 16        # 49 idx columns per image
    KT = DP // 128        # 10 classify contraction tiles
    CHUNKS = [(0, 512), (512, 512), (1024, DP - 1024)]  # psum-bank chunks

    with tile.TileContext(nc) as tc:
        with (
            tc.tile_pool(name="const", bufs=1) as cpool,
            tc.tile_pool(name="gath", bufs=1) as gpool,
            tc.tile_pool(name="prod", bufs=1) as ppool,
            tc.tile_pool(name="misc", bufs=1) as mpool,
            tc.tile_pool(name="psum", bufs=1, space="PSUM") as psum,
            tc.tile_pool(name="psumt", bufs=2, space="PSUM") as psumt,
        ):
            idx_sb = cpool.tile([128, BATCH * NIDX], i16)
            nc.sync.dma_start(idx_sb[:], idxw.ap())
            posx_sb = cpool.tile([128, PBLK * DP // 2], u16)
            nc.sync.dma_start(posx_sb[:], posx.ap())
            sel_sb = cpool.tile([128, BATCH * 2 * BATCH], fp8)
            nc.sync.dma_start(sel_sb[:], selw.ap())
            cls_sb = cpool.tile([128, KT * C], fp32)
            nc.sync.dma_start(cls_sb[:], clsw.ap())
            id_sb = cpool.tile([BATCH, BATCH], bf16)
            nc.sync.dma_start(id_sb[:], identw.ap())

            bund = psum.tile([BATCH, DP], fp32)

            NGBUF = 6
            NPBUF = 3
            g_tiles = [gpool.tile([128, PBLK * DP], fp8, name=f"g{i}")
                       for i in range(NGBUF)]
            pr_tiles = [ppool.tile([128, PBLK * DP], fp8, name=f"pr{i}")
                        for i in range(NPBUF)]

            # rows 784..1023 are never written by the gather: zero them once
            # so the padded XOR/matmul tail contributes exact zeros
            for t in g_tiles:
                t3 = t[:].rearrange("p (n m) -> p n m", m=DP)
                nc.gpsimd.memset(t3[:, 6:8, :], 0)

            for b in range(BATCH):
                g = g_tiles[b % NGBUF]
                pr = pr_tiles[b % NPBUF]
                g3 = g[:].rearrange("p (n m) -> p n m", m=DP)
                pr3 = pr[:].rearrange("p (n m) -> p n m", m=DP)

                nc.gpsimd.dma_gather(
                    g3[:, :GBLK, :], lvl.ap(),
                    idx_sb[:, b * NIDX:(b + 1) * NIDX],
                    num_idxs=P, num_idxs_reg=P, elem_size=DP,
                    queue_num=b % 4,
                )
                # bind: pos * lvl for +-1 values == XOR of position sign bits
                nc.vector.tensor_tensor(
                    pr[:].bitcast(u16), g[:].bitcast(u16), posx_sb[:],
                    op=mybir.AluOpType.bitwise_xor,
                )

                sel_b = sel_sb[:, b * 2 * BATCH:(b + 1) * 2 * BATCH]
                sel3 = sel_b.rearrange("p (t m) -> p t m", t=2)
                for j in range(PBLK // 2):
                    for (c0, cn) in CHUNKS:
                        nc.tensor.matmul(
                            bund[:, c0:c0 + cn],
                            sel3,
                            pr3[:, 2 * j:2 * j + 2, c0:c0 + cn],
                            start=(b == 0 and j == 0),
                            stop=(b == BATCH - 1 and j == PBLK // 2 - 1),
                            perf_mode=mybir.MatmulPerfMode.DoubleRow,
                        )

            # sign (integer sums; -0.5 bias makes where(x>0,1,-1) exact)
            bias_t = mpool.tile([BATCH, 1], fp32)
            nc.gpsimd.memset(bias_t[:], -0.5)
            enc = mpool.tile([BATCH, DP], bf16)
            nc.scalar.activation(enc[:], bund[:],
                                 mybir.ActivationFunctionType.Sign,
                                 bias=bias_t[:])

            # classify: transpose 128-col chunks of enc, then fp32 matmul
            logit_ps = psum.tile([C, BATCH], fp32)
            for kt in range(KT):
                tp = psumt.tile([128, BATCH], bf16, name="tp")
                nc.tensor.transpose(tp[:], enc[:, kt * 128:(kt + 1) * 128],
                                    id_sb[:])
                etc = mpool.tile([128, BATCH], fp32, name="etc", bufs=2)
                nc.scalar.copy(etc[:], tp[:])
                nc.tensor.matmul(
                    logit_ps[:], cls_sb[:, kt * C:(kt + 1) * C], etc[:],
                    start=(kt == 0), stop=(kt == KT - 1),
                )

            logit_sb = mpool.tile([C, BATCH], fp32)
            nc.scalar.copy(logit_sb[:], logit_ps[:])
            nc.sync.dma_start(out.ap(), logit_sb[:])

    nc.compile()
    return nc


def _prep_inputs(x, position, level_weight, classify_weight):
    """Host-side shard prep: returns in_maps for the 8 cores."""
    xf = x.reshape(BATCH, P).astype(np.float32)
    idx = np.clip(np.round(xf * np.float32(L - 1)), 0, L - 1).astype(np.int16)
    # dma_gather wraps indices as [16, n/16]: index j at [j%16, j//16],
    # replicated across all 128 partitions
    idxw = np.ascontiguousarray(
        idx.reshape(BATCH, P // 16, 16).transpose(2, 0, 1)
    ).reshape(16, BATCH * (P // 16))
    idxw = np.tile(idxw, (8, 1))  # [128, ...]

    # one-hot batch selectors, duplicated on both DoubleRow K-planes
    sel = np.zeros((128, BATCH, 2, BATCH), np.float32)
    for b in range(BATCH):
        sel[:, b, :, b] = 1.0
    selw = sel.reshape(128, BATCH * 2 * BATCH).astype(ml_dtypes.float8_e4m3)

    identw = np.eye(BATCH, dtype=np.float32).astype(ml_dtypes.bfloat16)

    KT = DP // 128
    in_maps = []
    for core in range(NCORES):
        cols = slice(core * DC, (core + 1) * DC)

        lvl = np.zeros((L, DP), ml_dtypes.float8_e4m3)
        lvl[:, :DC] = level_weight[:, cols].astype(ml_dtypes.float8_e4m3)

        # position sign bits, gather-layout [part, blk, d], packed as u16
        pos = np.zeros((PBLK * 128, DP), np.float32)
        pos[:P, :DC] = position[:, cols]
        signs = (pos < 0).astype(np.uint8) << 7
        posx = np.ascontiguousarray(
            signs.reshape(PBLK, 128, DP).transpose(1, 0, 2)
        ).reshape(128, PBLK * DP).view(np.uint16)

        cls = np.zeros((C, DP), np.float32)
        cls[:, :DC] = classify_weight[:, cols]
        clsw = np.ascontiguousarray(
            cls.reshape(C, KT, 128).transpose(2, 1, 0)
        ).reshape(128, KT * C)

        in_maps.append({
            "lvl": lvl,
            "posx": posx,
            "selw": selw,
            "clsw": clsw,
            "idxw": idxw,
            "identw": identw,
        })
    return in_maps


def kernel(x, position, level_weight, classify_weight, _run_kwargs=None):
    global _compiled
    if _compiled is None:
        _compiled = _build_bass()
    nc = _compiled

    import concourse.bass_utils as bass_utils

    in_maps = _prep_inputs(x, position, level_weight, classify_weight)
    res = bass_utils.run_bass_kernel_spmd(
        nc, in_maps, core_ids=list(range(NCORES)), **(_run_kwargs or {})
    )
    logit = np.zeros((BATCH, C), np.float32)
    for core in range(NCORES):
        logit += res.results[core]["logitT"].T.astype(np.float32)
    kernel.last_result = res
    return logit
